# revision 41
# baseline (speedup 1.0000x reference)
"""Trainium2 Bass kernel for nn_MoEClassifier (dense-MoE classifier).

Data-parallel over 8 NeuronCores: batch B=16384 split into 8 shards of 2048;
all expert parameters replicated (~4 MB). Per core:

  phase 1 (per 1024-row half, pipelined with input DMA): LayerNorm stats in
           [b, d] layout; var via E[x^2]-mu^2 (ACT Square+accum);
           rstd via ACT Abs_reciprocal_sqrt + one Newton polish;
           fused xhat = x*rstd - mu*rstd in one tensor_scalar pass;
           PE-transpose to xhatT [d, b] (fp32 for the router, f32r for GEMM1).
  phase 2: router matmul (fp32 lhsT=xhatT) reconstructing x@rw via
           gate = sigma*(xhat@rw) + mu*colsum(rw) + rb;
           GEMM1 h^T = w1'.T @ xhat^T in f32r (ln affine folded into w1/b1 on
           host; weights DMA'd per expert-quartet), GELU+bias on ACT -> bf16;
           GEMM2 logits^T = w2.T @ h^T in bf16, 4 experts col-tiled per PSUM
           (w2 zero-padded to M=32), compacted to [80, 512] tiles via GPSIMD
           SBUF->SBUF DMAs.
  phase 3 (per expert-octet and 512-row group, overlapping phase 2's tail):
           exp(logits + b2) in transposed layout (per-partition bias),
           PE-transpose back to [b, e*c], per-expert softmax normalization,
           gate softmax + top-4 (vector.max8) + threshold mask, market mix.
           Exp ops are order-pinned after the Gelu blocks to avoid ACT
           table-set thrash.

Top-k note: selection via 4th-max threshold; gate probs are continuous so
exact ties have measure zero. Softmax without max-subtraction (|logits| small
by construction of the problem's scales).
"""
import sys
import os

sys.path.insert(0, "/opt/trn_rl_repo")

import numpy as np
import ml_dtypes

import concourse.bass as bass  # noqa: F401
from concourse import bacc
import concourse.mybir as mybir
from concourse.tile import TileContext, add_dep_helper
from concourse.bass_utils import run_bass_kernel_spmd
from concourse.masks import make_identity

F32 = mybir.dt.float32
F32R = mybir.dt.float32r
BF16 = mybir.dt.bfloat16
AF = mybir.ActivationFunctionType
ALU = mybir.AluOpType
AX = mybir.AxisListType

B, D, E, TOPK, C = 16384, 256, 16, 4, 10
EPS = 1e-5
NCORES = 8
BL = B // NCORES          # 2048 rows per core
NBT = BL // 128           # 16 row-tiles of 128
NBH = NBT // 2            # 8 row-tiles per half
KC = D // 128             # 2 contraction chunks
DC = D // 128             # 2 dout chunks
EC = E * C                # 160

_CACHE: dict = {}


def _build():
    nc = bacc.Bacc("TRN2", target_bir_lowering=False, debug=False, num_devices=NCORES)

    x_d = nc.declare_dram_parameter("x", [BL, D], F32, isOutput=False)
    w1_d = nc.declare_dram_parameter("w1p", [128, 4, KC, 4, DC, 128], F32R, isOutput=False)
    b1_d = nc.declare_dram_parameter("b1p", [128, DC, E], F32, isOutput=False)
    w2_d = nc.declare_dram_parameter("w2p", [128, KC, E, 32], BF16, isOutput=False)
    b2_d = nc.declare_dram_parameter("b2p", [80, 2], F32, isOutput=False)
    rw_d = nc.declare_dram_parameter("rwp", [128, KC, E], F32, isOutput=False)
    r1_d = nc.declare_dram_parameter("r1v", [E], F32, isOutput=False)
    rb_d = nc.declare_dram_parameter("rbv", [E], F32, isOutput=False)

    allp_d = nc.declare_dram_parameter("allp", [BL, EC], F32, isOutput=True)
    market_d = nc.declare_dram_parameter("market", [BL, C], F32, isOutput=True)
    gate_d = nc.declare_dram_parameter("gate", [BL, E], F32, isOutput=True)

    x_dv = x_d.rearrange("(bt p) d -> p bt d", p=128)
    allp_dv = allp_d.rearrange("(bt p) ec -> p bt ec", p=128)
    market_dv = market_d.rearrange("(bt p) c -> p bt c", p=128)
    gate_dv = gate_d.rearrange("(bt p) e -> p bt e", p=128)

    with TileContext(nc) as tc:
        with (
            tc.tile_pool(name="cp", bufs=1) as cp,
            tc.tile_pool(name="hp", bufs=5) as hp,
            tc.tile_pool(name="scr", bufs=2) as scr,
            tc.tile_pool(name="g1", bufs=2, space="PSUM") as g1,
            tc.tile_pool(name="g2", bufs=2, space="PSUM") as g2,
            tc.tile_pool(name="gp", bufs=2, space="PSUM") as gp,
        ):
            # ---------- input DMAs (x per quarter first; w1 per expert-quartet) ----------
            x_sb = [cp.tile([128, 4, D], F32, name=f"x_sb{qt}") for qt in range(4)]
            for qt in range(4):
                nc.sync.dma_start(x_sb[qt][:], x_dv[:, 4 * qt : 4 * (qt + 1)])
            b1sb = cp.tile([128, DC, E], F32, name="b1sb")
            nc.sync.dma_start(b1sb[:], b1_d[:])
            w2sb = cp.tile([128, KC, E, 32], BF16, name="w2sb")
            nc.sync.dma_start(w2sb[:], w2_d[:])
            b2sb = cp.tile([80, 2], F32, name="b2sb")
            nc.sync.dma_start(b2sb[:], b2_d[:])
            rwsb = cp.tile([128, KC, E], F32, name="rwsb")
            nc.sync.dma_start(rwsb[:], rw_d[:])
            r1b = cp.tile([128, E], F32, name="r1b")
            nc.sync.dma_start(r1b[:], r1_d[None, :].to_broadcast((128, E)))
            rbb = cp.tile([128, E], F32, name="rbb")
            nc.sync.dma_start(rbb[:], rb_d[None, :].to_broadcast((128, E)))
            ident = cp.tile([128, 128], F32, name="ident")
            make_identity(nc, ident[:])
            w1sb = [cp.tile([128, KC, 4, DC, 128], F32R, name=f"w1sb{q}") for q in range(4)]
            for q in range(4):
                nc.sync.dma_start(w1sb[q][:], w1_d[:, q])

            # ---------- phase 1 (per 512-row quarter) ----------
            xhatT = [cp.tile([128, KC, BL // 2], F32, name=f"xhatT{h}") for h in range(2)]
            xhatTr = [cp.tile([128, KC, 512], F32R, name=f"xhatTr{qt}") for qt in range(4)]
            mu_q, sigma_q = [], []
            for qt in range(4):
                xh = x_sb[qt]
                sums = cp.tile([128, 4], F32, name=f"sums{qt}")
                nc.vector.reduce_sum(sums[:], xh[:], axis=AX.X)
                mu = cp.tile([128, 4], F32, name=f"mu{qt}")
                nc.vector.tensor_scalar(mu[:], sums[:], 1.0 / D, None, op0=ALU.mult)
                ssq = cp.tile([128, 4], F32, name=f"ssq{qt}")
                for bt in range(4):
                    sq_scr = scr.tile([128, D], F32, tag="sq_scr")
                    nc.scalar.activation(
                        sq_scr[:], xh[:, bt], AF.Square, accum_out=ssq[:, bt : bt + 1]
                    )
                vareps = cp.tile([128, 4], F32, name=f"vareps{qt}")
                nc.vector.tensor_scalar(vareps[:], ssq[:], 1.0 / D, EPS, op0=ALU.mult, op1=ALU.add)
                musq = cp.tile([128, 4], F32, name=f"musq{qt}")
                nc.vector.tensor_mul(musq[:], mu[:], mu[:])
                nc.vector.tensor_sub(vareps[:], vareps[:], musq[:])
                # rstd = rsqrt(vareps) via ACT table + one Newton polish
                rstd = cp.tile([128, 4], F32, name=f"rstd{qt}")
                nc.scalar.activation(rstd[:], vareps[:], AF.Abs_reciprocal_sqrt)
                nt = cp.tile([128, 4], F32, name=f"nt{qt}")
                nc.vector.tensor_mul(nt[:], rstd[:], rstd[:])
                nc.vector.tensor_mul(nt[:], nt[:], vareps[:])
                nc.vector.tensor_scalar(nt[:], nt[:], -0.5, 1.5, op0=ALU.mult, op1=ALU.add)
                nc.vector.tensor_mul(rstd[:], rstd[:], nt[:])
                sigma = cp.tile([128, 4], F32, name=f"sigma{qt}")
                nc.vector.tensor_mul(sigma[:], vareps[:], rstd[:])
                negmurstd = cp.tile([128, 4], F32, name=f"negmurstd{qt}")
                nc.vector.tensor_mul(negmurstd[:], mu[:], rstd[:])
                nc.vector.tensor_scalar(negmurstd[:], negmurstd[:], -1.0, None, op0=ALU.mult)
                mu_q.append(mu)
                sigma_q.append(sigma)

                for bt in range(4):
                    nc.vector.tensor_scalar(
                        xh[:, bt], xh[:, bt],
                        rstd[:, bt : bt + 1], negmurstd[:, bt : bt + 1],
                        op0=ALU.mult, op1=ALU.add,
                    )
                h = qt // 2
                pt = g1.tile([128, 1024], F32, tag="g1t")
                for kc in range(KC):
                    for bt in range(4):
                        nc.tensor.transpose(
                            pt[:, 512 * kc + 128 * bt : 512 * kc + 128 * (bt + 1)],
                            xh[:, bt, 128 * kc : 128 * (kc + 1)],
                            ident[:],
                        )
                nc.vector.tensor_copy(xhatTr[qt][:], pt[:])
                nc.scalar.copy(
                    xhatT[h].rearrange("p kc (hf c) -> p kc hf c", hf=2)[:, :, qt % 2, :],
                    pt.rearrange("p (kc c) -> p kc c", kc=KC),
                )

            # ---------- phase 2a: router ----------
            glog = cp.tile([128, NBT, E], F32, name="glog")
            for qt in range(4):
                h = qt // 2
                gate_pre = cp.tile([128, 4, E], F32, name=f"gate_pre{qt}")
                pg = gp.tile([128, 4 * E], F32, tag="gpt")
                for bt in range(4):
                    for kc in range(KC):
                        nc.tensor.matmul(
                            pg[:, E * bt : E * (bt + 1)],
                            xhatT[h][:, kc, 512 * (qt % 2) + 128 * bt : 512 * (qt % 2) + 128 * (bt + 1)],
                            rwsb[:, kc],
                            start=(kc == 0),
                            stop=(kc == KC - 1),
                        )
                nc.vector.tensor_copy(gate_pre[:], pg[:])
                gl = glog[:, 4 * qt : 4 * (qt + 1)]
                gterm = cp.tile([128, 4, E], F32, name=f"gterm{qt}")
                nc.vector.tensor_mul(
                    gterm[:],
                    mu_q[qt][:, :, None].to_broadcast((128, 4, E)),
                    r1b[:, None, :].to_broadcast((128, 4, E)),
                )
                nc.vector.tensor_add(gterm[:], gterm[:], rbb[:, None, :].to_broadcast((128, 4, E)))
                nc.vector.tensor_mul(gl, gate_pre[:], sigma_q[qt][:, :, None].to_broadcast((128, 4, E)))
                nc.vector.tensor_add(gl, gl, gterm[:])

            # ---------- phase 2b: experts ----------
            lgT = [[cp.tile([80, 512], F32, name=f"lgT{g}_{bt}") for bt in range(4)] for g in range(2)]
            gelu_insts = []

            def emit_gemm1(q):
                hts = []
                for j in range(4):
                    e = 4 * q + j
                    ht = hp.tile([128, KC, BL], BF16, tag="hT")
                    hts.append(ht)
                    for dc in range(DC):
                        for half in range(2):
                            ps = g1.tile([128, 1024], F32, tag="g1t")
                            for nh in range(2):
                                col = 512 * nh
                                for kc in range(KC):
                                    nc.tensor.matmul(
                                        ps[:, col : col + 512],
                                        w1sb[q][:, kc, j, dc],
                                        xhatTr[2 * half + nh][:, kc, :],
                                        start=(kc == 0),
                                        stop=(kc == KC - 1),
                                    )
                            gelu_insts.append(
                                nc.scalar.activation(
                                    ht[:, dc, 1024 * half : 1024 * (half + 1)],
                                    ps[:],
                                    AF.Gelu,
                                    bias=b1sb[:, dc, e : e + 1],
                                )
                            )
                return hts

            def emit_gemm2(q, hts):
                goct = q // 2
                rowbase = 40 * (q % 2)
                for bt in range(4):
                    psq = g2.tile([128, 512], F32, tag="g2t")
                    for j in range(4):
                        e = 4 * q + j
                        for kc in range(KC):
                            nc.tensor.matmul(
                                psq[32 * j : 32 * j + 32, :],
                                w2sb[:, kc, e],
                                hts[j][:, kc, 512 * bt : 512 * (bt + 1)],
                                start=(kc == 0),
                                stop=(kc == KC - 1),
                                tile_position=(0, 32 * j),
                            )
                    lg_scr = scr.tile([128, 512], F32, tag="lg_scr")
                    nc.vector.tensor_copy(lg_scr[:], psq[:])
                    for j in range(4):
                        nc.gpsimd.dma_start(
                            lgT[goct][bt][rowbase + C * j : rowbase + C * (j + 1), :],
                            lg_scr[32 * j : 32 * j + C, :],
                        )

            for q in range(E // 4):
                emit_gemm2(q, emit_gemm1(q))

            # ---------- router + gate (PE gap-fill; needed only for wsel) ----------
            # ---------- phase 3 ----------
            # gate softmax + top-4 weights
            gexp = cp.tile([128, NBT, E], F32, name="gexp")
            gexp_i = nc.scalar.activation(gexp[:], glog[:], AF.Exp)
            gsum = cp.tile([128, NBT], F32, name="gsum")
            nc.vector.reduce_sum(gsum[:], gexp[:], axis=AX.X)
            grec = cp.tile([128, NBT], F32, name="grec")
            nc.vector.reciprocal(grec[:], gsum[:])
            gprob = cp.tile([128, NBT, E], F32, name="gprob")
            nc.vector.tensor_mul(gprob[:], gexp[:], grec[:, :, None].to_broadcast((128, NBT, E)))
            nc.sync.dma_start(gate_dv, gprob[:])

            m8 = cp.tile([128, NBT, 8], F32, name="m8")
            for bt in range(NBT):
                nc.vector.max(out=m8[:, bt], in_=gprob[:, bt])
            t4sum = cp.tile([128, NBT], F32, name="t4sum")
            nc.vector.reduce_sum(t4sum[:], m8[:, :, 0:TOPK], axis=AX.X)
            t4rec = cp.tile([128, NBT], F32, name="t4rec")
            nc.vector.reciprocal(t4rec[:], t4sum[:])
            wsel = cp.tile([128, NBT, E], F32, name="wsel")
            nc.vector.tensor_tensor(
                wsel[:], gprob[:],
                m8[:, :, TOPK - 1 : TOPK].to_broadcast((128, NBT, E)),
                op=ALU.is_ge,
            )
            nc.vector.tensor_mul(wsel[:], wsel[:], gprob[:])
            nc.vector.tensor_mul(wsel[:], wsel[:], t4rec[:, :, None].to_broadcast((128, NBT, E)))

            # per-octet expert softmax + market mix; octet 0 unblocks after
            # quartet 2's last gelu, octet 1 (+ gate exp) after the last gelu.
            expT = [[cp.tile([80, 512], F32, name=f"expT{g}_{bt}") for bt in range(4)] for g in range(2)]
            nat = [[cp.tile([128, 4, 80], F32, name=f"nat{g}_{bt}") for bt in range(4)] for g in range(2)]
            mk_part = [[cp.tile([128, 4, C], F32, name=f"mk{g}_{bt}") for bt in range(4)] for g in range(2)]
            exp_holds = {0: gelu_insts[47], 1: gelu_insts[63]}
            add_dep_helper(gexp_i.ins, gelu_insts[47].ins, sync=False, reason="act-table order")
            for g in range(2):
                for bt4 in range(4):
                    xi = nc.scalar.activation(
                        expT[g][bt4][:], lgT[g][bt4][:], AF.Exp, bias=b2sb[:, g : g + 1]
                    )
                    add_dep_helper(xi.ins, exp_holds[g].ins, sync=False, reason="act-table order")
                    pt = g1.tile([128, 1024], F32, tag="g1t")
                    for sub in range(4):
                        nc.tensor.transpose(
                            pt[:, 80 * sub : 80 * (sub + 1)],
                            expT[g][bt4][:, 128 * sub : 128 * (sub + 1)],
                            ident[:80, :80],
                        )
                    nc.scalar.copy(nat[g][bt4][:], pt[:, :320])
                    nat3 = nat[g][bt4].rearrange("p s (e c) -> p s e c", c=C)
                    pse = cp.tile([128, 4, 8], F32, name=f"pse{g}_{bt4}")
                    nc.vector.reduce_sum(pse[:], nat3, axis=AX.X)
                    prec = cp.tile([128, 4, 8], F32, name=f"prec{g}_{bt4}")
                    nc.vector.reciprocal(prec[:], pse[:])
                    probs = cp.tile([128, 4, 80], F32, name=f"probs{g}_{bt4}")
                    nc.vector.tensor_mul(
                        probs.rearrange("p s (e c) -> p s e c", c=C),
                        nat3,
                        prec[:, :, :, None].to_broadcast((128, 4, 8, C)),
                    )
                    nc.sync.dma_start(
                        allp_dv[:, 4 * bt4 : 4 * (bt4 + 1), 80 * g : 80 * (g + 1)],
                        probs[:],
                    )
                    wprob = cp.tile([128, 4, 80], F32, name=f"wprob{g}_{bt4}")
                    nc.vector.tensor_mul(
                        wprob.rearrange("p s (e c) -> p s e c", c=C),
                        probs.rearrange("p s (e c) -> p s e c", c=C),
                        wsel[:, 4 * bt4 : 4 * (bt4 + 1), 8 * g : 8 * (g + 1), None].to_broadcast(
                            (128, 4, 8, C)
                        ),
                    )
                    nc.vector.reduce_sum(
                        mk_part[g][bt4][:],
                        wprob.rearrange("p s (e c) -> p s c e", c=C),
                        axis=AX.X,
                    )
            for bt4 in range(4):
                market = cp.tile([128, 4, C], F32, name=f"market{bt4}")
                nc.vector.tensor_add(market[:], mk_part[0][bt4][:], mk_part[1][bt4][:])
                nc.sync.dma_start(market_dv[:, 4 * bt4 : 4 * (bt4 + 1)], market[:])

    nc.finalize()
    return nc


def _prep_inputs(features, router_w, router_b, ln_g, ln_b, w1, b1, w2, b2):
    f32 = np.float32
    x = np.ascontiguousarray(np.asarray(features, f32))
    rw = np.asarray(router_w, f32)
    rb = np.asarray(router_b, f32)
    g = np.asarray(ln_g, f32)
    lb = np.asarray(ln_b, f32)
    w1 = np.asarray(w1, f32)
    b1 = np.asarray(b1, f32)
    w2 = np.asarray(w2, f32)
    b2 = np.asarray(b2, f32)

    # fold LN affine into expert weights: xn@w1+b1 = xhat@(g*w1) + (b1 + lb@w1)
    w1f = g[:, :, None] * w1                      # [E, D, D]
    b1f = b1 + np.einsum("ed,edh->eh", lb, w1)    # [E, D]

    # [kp, q(=e//4), kc, j(=e%4), dc, m]
    w1p = np.ascontiguousarray(
        w1f.reshape(4, 4, KC, 128, DC, 128).transpose(3, 0, 2, 1, 4, 5)
    )
    b1p = np.ascontiguousarray(b1f.reshape(E, DC, 128).transpose(2, 1, 0))
    w2pad = np.zeros((E, D, 32), np.float32)
    w2pad[:, :, :C] = w2
    w2p = np.ascontiguousarray(
        w2pad.reshape(E, KC, 128, 32).transpose(2, 1, 0, 3).astype(ml_dtypes.bfloat16)
    )
    b2p = np.ascontiguousarray(b2.reshape(2, 80).T)
    rwp = np.ascontiguousarray(rw.reshape(KC, 128, E).transpose(1, 0, 2))
    r1v = np.ascontiguousarray(rw.sum(axis=0))
    rbv = np.ascontiguousarray(rb)

    shared = {"w1p": w1p, "b1p": b1p, "w2p": w2p, "b2p": b2p, "rwp": rwp, "r1v": r1v, "rbv": rbv}
    return [dict(shared, x=np.ascontiguousarray(x[i * BL : (i + 1) * BL])) for i in range(NCORES)]


def run(inputs: dict, trace: bool = False):
    if "nc" not in _CACHE:
        _CACHE["nc"] = _build()
    nc = _CACHE["nc"]
    in_maps = _prep_inputs(**inputs)
    res = run_bass_kernel_spmd(nc, in_maps, list(range(NCORES)), trace=trace)
    market = np.concatenate([r["market"] for r in res.results], axis=0)
    gate = np.concatenate([r["gate"] for r in res.results], axis=0)
    allp_nat = np.concatenate([r["allp"] for r in res.results], axis=0).reshape(B, E, C)
    all_probs = np.ascontiguousarray(allp_nat.transpose(1, 0, 2))
    return (market, all_probs, gate), res


def kernel(**inputs):
    out, _ = run(inputs, trace=False)
    return out


if __name__ == "__main__":
    rng = np.random.default_rng(0)
    demo = {
        "features": rng.standard_normal((B, D)).astype(np.float32),
        "router_w": (rng.standard_normal((D, E)) * 0.02).astype(np.float32),
        "router_b": np.zeros(E, np.float32),
        "ln_g": np.ones((E, D), np.float32),
        "ln_b": np.zeros((E, D), np.float32),
        "w1": (rng.standard_normal((E, D, D)) * 0.02).astype(np.float32),
        "b1": np.zeros((E, D), np.float32),
        "w2": (rng.standard_normal((E, D, C)) * 0.02).astype(np.float32),
        "b2": np.zeros((E, C), np.float32),
    }
    out, res = run(demo)
    print("ran ok", [o.shape for o in out], "exec_time_ns:", res.exec_time_ns)


# revision 42
# speedup vs baseline: 1.0128x; 1.0128x over previous
"""Trainium2 Bass kernel for nn_MoEClassifier (dense-MoE classifier).

Data-parallel over 8 NeuronCores: batch B=16384 split into 8 shards of 2048;
all expert parameters replicated (~4 MB). Per core:

  phase 1 (per 1024-row half, pipelined with input DMA): LayerNorm stats in
           [b, d] layout; var via E[x^2]-mu^2 (ACT Square+accum);
           rstd via ACT Abs_reciprocal_sqrt + one Newton polish;
           fused xhat = x*rstd - mu*rstd in one tensor_scalar pass;
           PE-transpose to xhatT [d, b] (fp32 for the router, f32r for GEMM1).
  phase 2: router matmul (fp32 lhsT=xhatT) reconstructing x@rw via
           gate = sigma*(xhat@rw) + mu*colsum(rw) + rb;
           GEMM1 h^T = w1'.T @ xhat^T in f32r (ln affine folded into w1/b1 on
           host; weights DMA'd per expert-quartet), GELU+bias on ACT -> bf16;
           GEMM2 logits^T = w2.T @ h^T in bf16, 4 experts col-tiled per PSUM
           (w2 zero-padded to M=32), compacted to [80, 512] tiles via GPSIMD
           SBUF->SBUF DMAs.
  phase 3 (per expert-octet and 512-row group, overlapping phase 2's tail):
           exp(logits + b2) in transposed layout (per-partition bias),
           PE-transpose back to [b, e*c], per-expert softmax normalization,
           gate softmax + top-4 (vector.max8) + threshold mask, market mix.
           Exp ops are order-pinned after the Gelu blocks to avoid ACT
           table-set thrash.

Top-k note: selection via 4th-max threshold; gate probs are continuous so
exact ties have measure zero. Softmax without max-subtraction (|logits| small
by construction of the problem's scales).
"""
import sys
import os

sys.path.insert(0, "/opt/trn_rl_repo")

import numpy as np
import ml_dtypes

import concourse.bass as bass  # noqa: F401
from concourse import bacc
import concourse.mybir as mybir
from concourse.tile import TileContext, add_dep_helper
from concourse.bass_utils import run_bass_kernel_spmd
from concourse.masks import make_identity

F32 = mybir.dt.float32
F32R = mybir.dt.float32r
BF16 = mybir.dt.bfloat16
AF = mybir.ActivationFunctionType
ALU = mybir.AluOpType
AX = mybir.AxisListType

B, D, E, TOPK, C = 16384, 256, 16, 4, 10
EPS = 1e-5
NCORES = 8
BL = B // NCORES          # 2048 rows per core
NBT = BL // 128           # 16 row-tiles of 128
NBH = NBT // 2            # 8 row-tiles per half
KC = D // 128             # 2 contraction chunks
DC = D // 128             # 2 dout chunks
EC = E * C                # 160

_CACHE: dict = {}


def _build():
    nc = bacc.Bacc("TRN2", target_bir_lowering=False, debug=False, num_devices=NCORES)

    x_d = nc.declare_dram_parameter("x", [BL, D], F32, isOutput=False)
    w1_d = nc.declare_dram_parameter("w1p", [128, 4, KC, 4, DC, 128], F32R, isOutput=False)
    b1_d = nc.declare_dram_parameter("b1p", [128, DC, E], F32, isOutput=False)
    w2_d = nc.declare_dram_parameter("w2p", [128, KC, E, 32], BF16, isOutput=False)
    b2_d = nc.declare_dram_parameter("b2p", [80, 2], F32, isOutput=False)
    rw_d = nc.declare_dram_parameter("rwp", [128, KC, E], F32, isOutput=False)
    r1_d = nc.declare_dram_parameter("r1v", [E], F32, isOutput=False)
    rb_d = nc.declare_dram_parameter("rbv", [E], F32, isOutput=False)

    allp_d = nc.declare_dram_parameter("allp", [BL, EC], F32, isOutput=True)
    market_d = nc.declare_dram_parameter("market", [BL, C], F32, isOutput=True)
    gate_d = nc.declare_dram_parameter("gate", [BL, E], F32, isOutput=True)

    x_dv = x_d.rearrange("(bt p) d -> p bt d", p=128)
    allp_dv = allp_d.rearrange("(bt p) ec -> p bt ec", p=128)
    market_dv = market_d.rearrange("(bt p) c -> p bt c", p=128)
    gate_dv = gate_d.rearrange("(bt p) e -> p bt e", p=128)

    with TileContext(nc) as tc:
        with (
            tc.tile_pool(name="cp", bufs=1) as cp,
            tc.tile_pool(name="hp", bufs=5) as hp,
            tc.tile_pool(name="scr", bufs=2) as scr,
            tc.tile_pool(name="g1", bufs=3, space="PSUM") as g1,
            tc.tile_pool(name="g2", bufs=1, space="PSUM") as g2,
            tc.tile_pool(name="gp", bufs=1, space="PSUM") as gp,
        ):
            # ---------- input DMAs (x per quarter first; w1 per expert-quartet) ----------
            x_sb = [cp.tile([128, 4, D], F32, name=f"x_sb{qt}") for qt in range(4)]
            for qt in range(4):
                nc.sync.dma_start(x_sb[qt][:], x_dv[:, 4 * qt : 4 * (qt + 1)])
            b1sb = cp.tile([128, DC, E], F32, name="b1sb")
            nc.sync.dma_start(b1sb[:], b1_d[:])
            w2sb = cp.tile([128, KC, E, 32], BF16, name="w2sb")
            nc.sync.dma_start(w2sb[:], w2_d[:])
            b2sb = cp.tile([80, 2], F32, name="b2sb")
            nc.sync.dma_start(b2sb[:], b2_d[:])
            rwsb = cp.tile([128, KC, E], F32, name="rwsb")
            nc.sync.dma_start(rwsb[:], rw_d[:])
            r1b = cp.tile([128, E], F32, name="r1b")
            nc.sync.dma_start(r1b[:], r1_d[None, :].to_broadcast((128, E)))
            rbb = cp.tile([128, E], F32, name="rbb")
            nc.sync.dma_start(rbb[:], rb_d[None, :].to_broadcast((128, E)))
            ident = cp.tile([128, 128], F32, name="ident")
            make_identity(nc, ident[:])
            w1sb = [cp.tile([128, KC, 4, DC, 128], F32R, name=f"w1sb{q}") for q in range(4)]
            for q in range(4):
                nc.sync.dma_start(w1sb[q][:], w1_d[:, q])

            # ---------- phase 1 (per 512-row quarter) ----------
            xhatT = [cp.tile([128, KC, BL // 2], F32, name=f"xhatT{h}") for h in range(2)]
            xhatTr = [cp.tile([128, KC, 512], F32R, name=f"xhatTr{qt}") for qt in range(4)]
            mu_q, sigma_q = [], []
            for qt in range(4):
                xh = x_sb[qt]
                sums = cp.tile([128, 4], F32, name=f"sums{qt}")
                nc.vector.reduce_sum(sums[:], xh[:], axis=AX.X)
                mu = cp.tile([128, 4], F32, name=f"mu{qt}")
                nc.vector.tensor_scalar(mu[:], sums[:], 1.0 / D, None, op0=ALU.mult)
                ssq = cp.tile([128, 4], F32, name=f"ssq{qt}")
                for bt in range(4):
                    sq_scr = scr.tile([128, D], F32, tag="sq_scr")
                    nc.scalar.activation(
                        sq_scr[:], xh[:, bt], AF.Square, accum_out=ssq[:, bt : bt + 1]
                    )
                vareps = cp.tile([128, 4], F32, name=f"vareps{qt}")
                nc.vector.tensor_scalar(vareps[:], ssq[:], 1.0 / D, EPS, op0=ALU.mult, op1=ALU.add)
                musq = cp.tile([128, 4], F32, name=f"musq{qt}")
                nc.vector.tensor_mul(musq[:], mu[:], mu[:])
                nc.vector.tensor_sub(vareps[:], vareps[:], musq[:])
                # rstd = rsqrt(vareps) via ACT table + one Newton polish
                rstd = cp.tile([128, 4], F32, name=f"rstd{qt}")
                nc.scalar.activation(rstd[:], vareps[:], AF.Abs_reciprocal_sqrt)
                nt = cp.tile([128, 4], F32, name=f"nt{qt}")
                nc.vector.tensor_mul(nt[:], rstd[:], rstd[:])
                nc.vector.tensor_mul(nt[:], nt[:], vareps[:])
                nc.vector.tensor_scalar(nt[:], nt[:], -0.5, 1.5, op0=ALU.mult, op1=ALU.add)
                nc.vector.tensor_mul(rstd[:], rstd[:], nt[:])
                sigma = cp.tile([128, 4], F32, name=f"sigma{qt}")
                nc.vector.tensor_mul(sigma[:], vareps[:], rstd[:])
                negmurstd = cp.tile([128, 4], F32, name=f"negmurstd{qt}")
                nc.vector.tensor_mul(negmurstd[:], mu[:], rstd[:])
                nc.vector.tensor_scalar(negmurstd[:], negmurstd[:], -1.0, None, op0=ALU.mult)
                mu_q.append(mu)
                sigma_q.append(sigma)

                for bt in range(4):
                    nc.vector.tensor_scalar(
                        xh[:, bt], xh[:, bt],
                        rstd[:, bt : bt + 1], negmurstd[:, bt : bt + 1],
                        op0=ALU.mult, op1=ALU.add,
                    )
                h = qt // 2
                pt = g1.tile([128, 1024], F32, tag="g1t")
                for kc in range(KC):
                    for bt in range(4):
                        nc.tensor.transpose(
                            pt[:, 512 * kc + 128 * bt : 512 * kc + 128 * (bt + 1)],
                            xh[:, bt, 128 * kc : 128 * (kc + 1)],
                            ident[:],
                        )
                nc.vector.tensor_copy(xhatTr[qt][:], pt[:])
                nc.scalar.copy(
                    xhatT[h].rearrange("p kc (hf c) -> p kc hf c", hf=2)[:, :, qt % 2, :],
                    pt.rearrange("p (kc c) -> p kc c", kc=KC),
                )

            # ---------- phase 2a: router ----------
            glog = cp.tile([128, NBT, E], F32, name="glog")
            for qt in range(4):
                h = qt // 2
                gate_pre = cp.tile([128, 4, E], F32, name=f"gate_pre{qt}")
                pg = gp.tile([128, 4 * E], F32, tag="gpt")
                for bt in range(4):
                    for kc in range(KC):
                        nc.tensor.matmul(
                            pg[:, E * bt : E * (bt + 1)],
                            xhatT[h][:, kc, 512 * (qt % 2) + 128 * bt : 512 * (qt % 2) + 128 * (bt + 1)],
                            rwsb[:, kc],
                            start=(kc == 0),
                            stop=(kc == KC - 1),
                        )
                nc.vector.tensor_copy(gate_pre[:], pg[:])
                gl = glog[:, 4 * qt : 4 * (qt + 1)]
                gterm = cp.tile([128, 4, E], F32, name=f"gterm{qt}")
                nc.vector.tensor_mul(
                    gterm[:],
                    mu_q[qt][:, :, None].to_broadcast((128, 4, E)),
                    r1b[:, None, :].to_broadcast((128, 4, E)),
                )
                nc.vector.tensor_add(gterm[:], gterm[:], rbb[:, None, :].to_broadcast((128, 4, E)))
                nc.vector.tensor_mul(gl, gate_pre[:], sigma_q[qt][:, :, None].to_broadcast((128, 4, E)))
                nc.vector.tensor_add(gl, gl, gterm[:])

            # ---------- phase 2b: experts ----------
            lgT = [[cp.tile([80, 512], F32, name=f"lgT{g}_{bt}") for bt in range(4)] for g in range(2)]
            gelu_insts = []

            def emit_gemm1(q):
                hts = []
                for j in range(4):
                    e = 4 * q + j
                    ht = hp.tile([128, KC, BL], BF16, tag="hT")
                    hts.append(ht)
                    for dc in range(DC):
                        for half in range(2):
                            ps = g1.tile([128, 1024], F32, tag="g1t")
                            for nh in range(2):
                                col = 512 * nh
                                for kc in range(KC):
                                    nc.tensor.matmul(
                                        ps[:, col : col + 512],
                                        w1sb[q][:, kc, j, dc],
                                        xhatTr[2 * half + nh][:, kc, :],
                                        start=(kc == 0),
                                        stop=(kc == KC - 1),
                                    )
                            gelu_insts.append(
                                nc.scalar.activation(
                                    ht[:, dc, 1024 * half : 1024 * (half + 1)],
                                    ps[:],
                                    AF.Gelu,
                                    bias=b1sb[:, dc, e : e + 1],
                                )
                            )
                return hts

            def emit_gemm2(q, hts):
                goct = q // 2
                rowbase = 40 * (q % 2)
                for bt in range(4):
                    psq = g2.tile([128, 512], F32, tag="g2t")
                    for j in range(4):
                        e = 4 * q + j
                        for kc in range(KC):
                            nc.tensor.matmul(
                                psq[32 * j : 32 * j + 32, :],
                                w2sb[:, kc, e],
                                hts[j][:, kc, 512 * bt : 512 * (bt + 1)],
                                start=(kc == 0),
                                stop=(kc == KC - 1),
                                tile_position=(0, 32 * j),
                            )
                    lg_scr = scr.tile([128, 512], F32, tag="lg_scr")
                    nc.vector.tensor_copy(lg_scr[:], psq[:])
                    for j in range(4):
                        nc.gpsimd.dma_start(
                            lgT[goct][bt][rowbase + C * j : rowbase + C * (j + 1), :],
                            lg_scr[32 * j : 32 * j + C, :],
                        )

            for q in range(E // 4):
                emit_gemm2(q, emit_gemm1(q))

            # ---------- router + gate (PE gap-fill; needed only for wsel) ----------
            # ---------- phase 3 ----------
            # gate softmax + top-4 weights
            gexp = cp.tile([128, NBT, E], F32, name="gexp")
            gexp_i = nc.scalar.activation(gexp[:], glog[:], AF.Exp)
            gsum = cp.tile([128, NBT], F32, name="gsum")
            nc.vector.reduce_sum(gsum[:], gexp[:], axis=AX.X)
            grec = cp.tile([128, NBT], F32, name="grec")
            nc.vector.reciprocal(grec[:], gsum[:])
            gprob = cp.tile([128, NBT, E], F32, name="gprob")
            nc.vector.tensor_mul(gprob[:], gexp[:], grec[:, :, None].to_broadcast((128, NBT, E)))
            nc.sync.dma_start(gate_dv, gprob[:])

            m8 = cp.tile([128, NBT, 8], F32, name="m8")
            for bt in range(NBT):
                nc.vector.max(out=m8[:, bt], in_=gprob[:, bt])
            t4sum = cp.tile([128, NBT], F32, name="t4sum")
            nc.vector.reduce_sum(t4sum[:], m8[:, :, 0:TOPK], axis=AX.X)
            t4rec = cp.tile([128, NBT], F32, name="t4rec")
            nc.vector.reciprocal(t4rec[:], t4sum[:])
            wsel = cp.tile([128, NBT, E], F32, name="wsel")
            nc.vector.tensor_tensor(
                wsel[:], gprob[:],
                m8[:, :, TOPK - 1 : TOPK].to_broadcast((128, NBT, E)),
                op=ALU.is_ge,
            )
            nc.vector.tensor_mul(wsel[:], wsel[:], gprob[:])
            nc.vector.tensor_mul(wsel[:], wsel[:], t4rec[:, :, None].to_broadcast((128, NBT, E)))

            # per-octet expert softmax + market mix; octet 0 unblocks after
            # quartet 2's last gelu, octet 1 (+ gate exp) after the last gelu.
            expT = [[cp.tile([80, 512], F32, name=f"expT{g}_{bt}") for bt in range(4)] for g in range(2)]
            nat = [[cp.tile([128, 4, 80], F32, name=f"nat{g}_{bt}") for bt in range(4)] for g in range(2)]
            mk_part = [[cp.tile([128, 4, C], F32, name=f"mk{g}_{bt}") for bt in range(4)] for g in range(2)]
            exp_holds = {0: gelu_insts[47], 1: gelu_insts[63]}
            add_dep_helper(gexp_i.ins, gelu_insts[47].ins, sync=False, reason="act-table order")
            for g in range(2):
                for bt4 in range(4):
                    xi = nc.scalar.activation(
                        expT[g][bt4][:], lgT[g][bt4][:], AF.Exp, bias=b2sb[:, g : g + 1]
                    )
                    add_dep_helper(xi.ins, exp_holds[g].ins, sync=False, reason="act-table order")
                    pt = g1.tile([128, 1024], F32, tag="g1t")
                    for sub in range(4):
                        nc.tensor.transpose(
                            pt[:, 80 * sub : 80 * (sub + 1)],
                            expT[g][bt4][:, 128 * sub : 128 * (sub + 1)],
                            ident[:80, :80],
                        )
                    nc.scalar.copy(nat[g][bt4][:], pt[:, :320])
                    nat3 = nat[g][bt4].rearrange("p s (e c) -> p s e c", c=C)
                    pse = cp.tile([128, 4, 8], F32, name=f"pse{g}_{bt4}")
                    nc.vector.reduce_sum(pse[:], nat3, axis=AX.X)
                    prec = cp.tile([128, 4, 8], F32, name=f"prec{g}_{bt4}")
                    nc.vector.reciprocal(prec[:], pse[:])
                    probs = cp.tile([128, 4, 80], F32, name=f"probs{g}_{bt4}")
                    nc.vector.tensor_mul(
                        probs.rearrange("p s (e c) -> p s e c", c=C),
                        nat3,
                        prec[:, :, :, None].to_broadcast((128, 4, 8, C)),
                    )
                    nc.sync.dma_start(
                        allp_dv[:, 4 * bt4 : 4 * (bt4 + 1), 80 * g : 80 * (g + 1)],
                        probs[:],
                    )
                    wprob = cp.tile([128, 4, 80], F32, name=f"wprob{g}_{bt4}")
                    nc.vector.tensor_mul(
                        wprob.rearrange("p s (e c) -> p s e c", c=C),
                        probs.rearrange("p s (e c) -> p s e c", c=C),
                        wsel[:, 4 * bt4 : 4 * (bt4 + 1), 8 * g : 8 * (g + 1), None].to_broadcast(
                            (128, 4, 8, C)
                        ),
                    )
                    nc.vector.reduce_sum(
                        mk_part[g][bt4][:],
                        wprob.rearrange("p s (e c) -> p s c e", c=C),
                        axis=AX.X,
                    )
            for bt4 in range(4):
                market = cp.tile([128, 4, C], F32, name=f"market{bt4}")
                nc.vector.tensor_add(market[:], mk_part[0][bt4][:], mk_part[1][bt4][:])
                nc.sync.dma_start(market_dv[:, 4 * bt4 : 4 * (bt4 + 1)], market[:])

    nc.finalize()
    return nc


def _prep_inputs(features, router_w, router_b, ln_g, ln_b, w1, b1, w2, b2):
    f32 = np.float32
    x = np.ascontiguousarray(np.asarray(features, f32))
    rw = np.asarray(router_w, f32)
    rb = np.asarray(router_b, f32)
    g = np.asarray(ln_g, f32)
    lb = np.asarray(ln_b, f32)
    w1 = np.asarray(w1, f32)
    b1 = np.asarray(b1, f32)
    w2 = np.asarray(w2, f32)
    b2 = np.asarray(b2, f32)

    # fold LN affine into expert weights: xn@w1+b1 = xhat@(g*w1) + (b1 + lb@w1)
    w1f = g[:, :, None] * w1                      # [E, D, D]
    b1f = b1 + np.einsum("ed,edh->eh", lb, w1)    # [E, D]

    # [kp, q(=e//4), kc, j(=e%4), dc, m]
    w1p = np.ascontiguousarray(
        w1f.reshape(4, 4, KC, 128, DC, 128).transpose(3, 0, 2, 1, 4, 5)
    )
    b1p = np.ascontiguousarray(b1f.reshape(E, DC, 128).transpose(2, 1, 0))
    w2pad = np.zeros((E, D, 32), np.float32)
    w2pad[:, :, :C] = w2
    w2p = np.ascontiguousarray(
        w2pad.reshape(E, KC, 128, 32).transpose(2, 1, 0, 3).astype(ml_dtypes.bfloat16)
    )
    b2p = np.ascontiguousarray(b2.reshape(2, 80).T)
    rwp = np.ascontiguousarray(rw.reshape(KC, 128, E).transpose(1, 0, 2))
    r1v = np.ascontiguousarray(rw.sum(axis=0))
    rbv = np.ascontiguousarray(rb)

    shared = {"w1p": w1p, "b1p": b1p, "w2p": w2p, "b2p": b2p, "rwp": rwp, "r1v": r1v, "rbv": rbv}
    return [dict(shared, x=np.ascontiguousarray(x[i * BL : (i + 1) * BL])) for i in range(NCORES)]


def run(inputs: dict, trace: bool = False):
    if "nc" not in _CACHE:
        _CACHE["nc"] = _build()
    nc = _CACHE["nc"]
    in_maps = _prep_inputs(**inputs)
    res = run_bass_kernel_spmd(nc, in_maps, list(range(NCORES)), trace=trace)
    market = np.concatenate([r["market"] for r in res.results], axis=0)
    gate = np.concatenate([r["gate"] for r in res.results], axis=0)
    allp_nat = np.concatenate([r["allp"] for r in res.results], axis=0).reshape(B, E, C)
    all_probs = np.ascontiguousarray(allp_nat.transpose(1, 0, 2))
    return (market, all_probs, gate), res


def kernel(**inputs):
    out, _ = run(inputs, trace=False)
    return out


if __name__ == "__main__":
    rng = np.random.default_rng(0)
    demo = {
        "features": rng.standard_normal((B, D)).astype(np.float32),
        "router_w": (rng.standard_normal((D, E)) * 0.02).astype(np.float32),
        "router_b": np.zeros(E, np.float32),
        "ln_g": np.ones((E, D), np.float32),
        "ln_b": np.zeros((E, D), np.float32),
        "w1": (rng.standard_normal((E, D, D)) * 0.02).astype(np.float32),
        "b1": np.zeros((E, D), np.float32),
        "w2": (rng.standard_normal((E, D, C)) * 0.02).astype(np.float32),
        "b2": np.zeros((E, C), np.float32),
    }
    out, res = run(demo)
    print("ran ok", [o.shape for o in out], "exec_time_ns:", res.exec_time_ns)
